# revision 1
# baseline (speedup 1.0000x reference)
"""Trainium2 Bass kernel for nn_DGProjectionBatchSparsity.

Computes: logits = x @ W.T (+b); per output neuron, mask of the top-k
(k=204) logits across the batch (4096). Output = mask (the straight-through
estimator output equals the mask numerically; the bias never changes the
ranking within a neuron column, so it is ignored).

Sharding: column-parallel over out_features — each of the 8 cores owns a
1024-neuron slab: GEMM -> [128 neurons x 4096 batch] tiles, per-partition
(per-neuron) exact top-k threshold via count-guided secant refinement plus
a single max8 finish, then mask = (logit >= T).

Algorithm per 128-neuron tile:
  1. PE: logits_t[o,b] in PSUM (f32), 8 chunks of 512 batch.
  2. ScalarE epilogue: copy PSUM->SBUF f32 + accumulate sum / sum-of-squares
     -> exact empirical mean/std per neuron.
  3. t0 = mu + z0*sigma (z0 = Phi^-1(1-204/4096)); 6 count passes
     (#{x >= t}, fused compare+accumulate on ScalarE/VectorE), with one
     Newton and four secant threshold updates.
  4. Final count c -> r = c-204 in [-8,7] for every neuron (validated);
     flip sign s so the true 204th value is among the 8 nearest on the
     deficient side; max8 over s*(x-t) with the opposite side pushed to
     -1e30; select the r-th candidate -> exact 204th largest value T.
  5. mask = (x >= T)  (exactly 204 ones per neuron).
"""

import math

import numpy as np

import concourse.bass as bass
import concourse.tile as tile
from concourse import mybir
from concourse.bass_utils import run_bass_kernel_spmd

# ---------------------------------------------------------------- constants
BATCH = 4096
IN = 512
OUT = 8192
NCORES = 8
OSHARD = OUT // NCORES          # 1024 neurons per core
NTILES = OSHARD // 128          # 8 o-tiles per core
KTILES = IN // 128              # 4 contraction tiles
BCHUNK = 512
NBCH = BATCH // BCHUNK          # 8 batch chunks
K = max(1, int(0.05 * BATCH))   # 204

Z0 = 1.6467503276689657                      # Phi^-1(1 - K/BATCH)
PHI_Z0 = math.exp(-0.5 * Z0 * Z0) / math.sqrt(2.0 * math.pi)
INV_SQRT2PI = 1.0 / math.sqrt(2.0 * math.pi)
NEG_BIG = -1.0e30

F32 = mybir.dt.float32
ALU = mybir.AluOpType
ACTF = mybir.ActivationFunctionType

N_COUNT_PASSES = 4   # counts at t0..t2, final count at t3
LOGITS_BUFS = 3
WORK_BUFS = 2
SMALL_BUFS = 2
PEN_MODE = "dve_stt"
EPI_PRIO_OFFSET = 120
SQ_PRIO_DELAY = 0
ZTILE_BUFS = 1
EPI_SPLIT = False
SPLIT_ACC = True
Z_ENGINE = "pool"
MASK_ENGINE = "pool"

DEBUG = False        # when True, o-tile 0 intermediates are DMA'd out

# -------------------------------------------- multi-wait split post-pass
# This container's walrus build lowers at most ONE semaphore wait per
# instruction (setupSyncWait asserts otherwise). Hoist extra waits onto
# same-engine NOPs inserted immediately before the instruction; per-engine
# program order makes this semantically identical.
from concourse.tile import TileContext
import bass_rust


def _split_multi_waits(nc):
    count = [0]

    def fresh():
        count[0] += 1
        return f"I-msw{count[0]}"

    for f in nc.m.functions:
        for bb in f.blocks:
            out = []
            changed = False
            for inst in bb.instructions:
                si = inst.sync_info
                if si is not None and si.on_wait and len(si.on_wait) > 1:
                    waits = list(si.on_wait)
                    for w in waits[:-1]:
                        nop = bass_rust.InstNoOp(name=fresh(), hint=None)
                        nop.engine = inst.engine
                        nop.sync_info = mybir.SyncInfo(on_wait=[w],
                                                       on_update=[])
                        out.append(nop)
                    si.on_wait = [waits[-1]]
                    changed = True
                out.append(inst)
            if changed:
                bb.instructions = out


# ---------------------------------------------------------------- program
def build_program():
    nc = bass.Bass("TRN2", target_bir_lowering=False, debug=False,
                   num_devices=NCORES)
    xT = nc.declare_dram_parameter("xT", [IN, BATCH], F32, isOutput=False)
    wT = nc.declare_dram_parameter("wT", [IN, OSHARD], F32, isOutput=False)
    mask_out = nc.declare_dram_parameter("mask", [OSHARD, BATCH], F32,
                                         isOutput=True)

    dbg = None
    if DEBUG:
        dbg = {
            "d_logits": nc.declare_dram_parameter("d_logits", [128, BATCH],
                                                  F32, isOutput=True),
            "d_ztile": nc.declare_dram_parameter("d_ztile", [128, BATCH],
                                                 F32, isOutput=True),
            "d_small": nc.declare_dram_parameter("d_small", [128, 64], F32,
                                                 isOutput=True),
        }

    with TileContext(nc) as tc:
        _emit(nc, tc, xT, wT, mask_out, dbg)
    _split_multi_waits(nc)
    return nc


def _emit(nc, tc, xT, wT, mask_out, dbg=None):
    import contextlib
    ctx = contextlib.ExitStack()
    with ctx:
        resident = ctx.enter_context(tc.tile_pool(name="resident", bufs=1))
        logits_p = ctx.enter_context(tc.tile_pool(name="logits",
                                                   bufs=LOGITS_BUFS))
        work_p = ctx.enter_context(tc.tile_pool(name="work", bufs=WORK_BUFS))
        small_p = ctx.enter_context(tc.tile_pool(name="small",
                                                 bufs=SMALL_BUFS))
        psum_p = ctx.enter_context(
            tc.tile_pool(name="psum", bufs=8, space="PSUM"))

        # ---- resident inputs
        xTr = xT.rearrange("(ko p) b -> p ko b", p=128)
        wTr = wT.rearrange("(ko p) o -> p ko o", p=128)
        xt = []
        wt = []
        for kt in range(KTILES):
            wk = resident.tile([128, OSHARD], F32, tag=f"wt{kt}",
                               name=f"wt{kt}")
            nc.sync.dma_start(wk[:], wTr[:, kt])
            wt.append(wk)
            xk = resident.tile([128, BATCH], F32, tag=f"xt{kt}",
                               name=f"xt{kt}")
            nc.sync.dma_start(xk[:], xTr[:, kt])
            xt.append(xk)
        iota16 = resident.tile([128, 32], F32, tag="iota16")
        for j in range(32):
            nc.vector.memset(iota16[:, j:j + 1], float(j))

        for ot in range(NTILES):
            _emit_tile(nc, tc, xt, wt, mask_out, ot,
                       logits_p, work_p, small_p, psum_p, iota16,
                       dbg if ot == 0 else None)


def _emit_tile(nc, tc, xt, wt, mask_out, ot, logits_p, work_p, small_p,
               psum_p, iota16, dbg=None):
    taps = []

    def tap(name, ap):
        if dbg is not None:
            taps.append((name, ap))

    v = nc.vector
    g = nc.gpsimd
    sc = nc.scalar

    logits = logits_p.tile([128, BATCH], F32, tag="logits")
    ztile = work_p.tile([128, BATCH], F32, tag="ztile", bufs=ZTILE_BUFS)
    maskt = work_p.tile([128, BATCH], F32, tag="maskt")

    if SPLIT_ACC:
        s1cl = [small_p.tile([128, 1], F32, tag=f"s1c{bc}", name=f"s1c{bc}")
                for bc in range(NBCH)]
        s2cl = [small_p.tile([128, 1], F32, tag=f"s2c{bc}", name=f"s2c{bc}")
                for bc in range(NBCH)]
    else:
        s1c = small_p.tile([128, NBCH], F32, tag="s1c")
        s2c = small_p.tile([128, NBCH], F32, tag="s2c")

    o_lo = ot * 128

    # ---- GEMM + epilogue per batch chunk
    pss = [psum_p.tile([128, BCHUNK], F32, tag="ps", name=f"ps{bc}")
           for bc in range(NBCH)]
    for bc in range(NBCH):
        for kt in range(KTILES):
            nc.tensor.matmul(
                pss[bc][:],
                wt[kt][:, o_lo:o_lo + 128],
                xt[kt][:, bc * BCHUNK:(bc + 1) * BCHUNK],
                start=(kt == 0),
                stop=(kt == KTILES - 1),
            )
    for bc in range(NBCH):
        b_lo = bc * BCHUNK
        # PSUM -> SBUF (+ per-chunk sums for the mean); high priority so
        # the PSUM bank frees promptly and the PE never stalls on banks.
        s1dst = s1cl[bc][:] if SPLIT_ACC else s1c[:, bc:bc + 1]
        s2dst = s2cl[bc][:] if SPLIT_ACC else s2c[:, bc:bc + 1]
        with tc.high_priority(offset=EPI_PRIO_OFFSET):
            sc.activation(logits[:, b_lo:b_lo + BCHUNK], pss[bc][:],
                          ACTF.Copy, accum_out=s1dst)
        # sum-of-squares from SBUF (does not hold the PSUM bank)
        sqc = small_p.tile([128, BCHUNK], F32, tag="sqc", name="sqc")
        with tc.high_priority(offset=-SQ_PRIO_DELAY):
            sc.activation(sqc[:], logits[:, b_lo:b_lo + BCHUNK], ACTF.Square,
                          accum_out=s2dst)

    def tiny(tag):
        return small_p.tile([128, 1], F32, tag=tag, name=tag)

    # ---- per-neuron stats
    if SPLIT_ACC:
        sgath = small_p.tile([128, 2 * NBCH], F32, tag="sgath")
        for bc in range(NBCH):
            v.tensor_copy(sgath[:, bc:bc + 1], s1cl[bc][:])
            v.tensor_copy(sgath[:, NBCH + bc:NBCH + bc + 1], s2cl[bc][:])
        S1 = tiny("S1")
        v.reduce_sum(S1[:], sgath[:, 0:NBCH], axis=mybir.AxisListType.X)
        S2 = tiny("S2")
        v.reduce_sum(S2[:], sgath[:, NBCH:], axis=mybir.AxisListType.X)
    else:
        S2 = tiny("S2")
        v.reduce_sum(S2[:], s2c[:], axis=mybir.AxisListType.X)
        S1 = tiny("S1")
        v.reduce_sum(S1[:], s1c[:], axis=mybir.AxisListType.X)
    mu = tiny("mu")
    v.tensor_scalar(mu[:], S1[:], 1.0 / BATCH, None, ALU.mult)
    var = tiny("var")
    mu2 = tiny("mu2")
    v.tensor_tensor(mu2[:], mu[:], mu[:], ALU.mult)
    v.tensor_scalar(var[:], S2[:], 1.0 / BATCH, None, ALU.mult)
    v.tensor_tensor(var[:], var[:], mu2[:], ALU.subtract)
    sig = tiny("sig")
    sc.activation(sig[:], var[:], ACTF.Sqrt)
    tap("S1", S1); tap("S2", S2); tap("mu", mu); tap("sig", sig)

    # t0 = mu + z0 * sigma ; initial slope ls0 = n*phi(z0)/sigma
    t_cur = tiny("t0")
    v.tensor_scalar(t_cur[:], sig[:], Z0, None, ALU.mult)
    v.tensor_tensor(t_cur[:], t_cur[:], mu[:], ALU.add)
    rsig = tiny("rsig")
    v.reciprocal(rsig[:], sig[:])
    ls = tiny("ls0")
    v.tensor_scalar(ls[:], rsig[:], BATCH * PHI_Z0, None, ALU.mult)
    tap("t0", t_cur)

    # ---- count passes (5): c0 ACT, c1 DVE, c2 ACT, c3 DVE, c4 DVE(final)
    def count_act(t_ap, tag):
        negt = tiny("negt" + tag)
        v.tensor_scalar(negt[:], t_ap[:], -1.0, None, ALU.mult)
        ssum = tiny("ssum" + tag)
        sc.activation(maskt[:], logits[:], ACTF.Sign, bias=negt[:],
                      accum_out=ssum[:])
        c = tiny("c" + tag)
        v.tensor_scalar(c[:], ssum[:], float(BATCH), 0.5, ALU.add, ALU.mult)
        return c

    def count_dve(t_ap, tag):
        c = tiny("cd" + tag)
        v.tensor_scalar(maskt[:], logits[:], t_ap[:], 0.0, ALU.is_ge,
                        ALU.add, accum_out=c[:])
        return c

    # pass 0 + Newton update using the gaussian slope
    c_prev = count_dve(t_cur, "p0")
    tap("c0", c_prev)
    t_prev = t_cur
    rls0 = tiny("rls0")
    v.reciprocal(rls0[:], ls[:])
    d0 = tiny("d0")
    v.tensor_scalar(d0[:], c_prev[:], -float(K), None, ALU.add)
    v.tensor_tensor(d0[:], d0[:], rls0[:], ALU.mult)
    t_cur = tiny("t1")
    v.tensor_tensor(t_cur[:], t_prev[:], d0[:], ALU.add)

    for i in range(1, N_COUNT_PASSES - 1):
        tag = f"p{i}"
        c_cur = count_dve(t_cur, tag)
        dc = tiny("dc" + tag)
        v.tensor_tensor(dc[:], c_cur[:], c_prev[:], ALU.subtract)
        dtn = tiny("dtn" + tag)  # t_prev - t_cur (negated dt)
        v.tensor_tensor(dtn[:], t_prev[:], t_cur[:], ALU.subtract)
        rdt = tiny("rdt" + tag)
        v.reciprocal(rdt[:], dtn[:])
        ssl = tiny("ssl" + tag)  # secant slope = dc/(t_prev-t_cur) >= 0
        v.tensor_tensor(ssl[:], dc[:], rdt[:], ALU.mult)
        dc2 = tiny("dc2" + tag)
        v.tensor_tensor(dc2[:], dc[:], dc[:], ALU.mult)
        sel = small_p.tile([128, 1], mybir.dt.uint32, tag="sel" + tag,
                           name="sel" + tag)
        v.tensor_scalar(sel[:], dc2[:], 9.0, None, ALU.is_ge)
        ls_new = tiny("ls" + tag)
        v.tensor_copy(ls_new[:], ls[:])
        v.copy_predicated(ls_new[:], sel[:], ssl[:])
        ls = ls_new
        rls = tiny("rls" + tag)
        v.reciprocal(rls[:], ls[:])
        step = tiny("step" + tag)
        v.tensor_scalar(step[:], c_cur[:], -float(K), None, ALU.add)
        v.tensor_tensor(step[:], step[:], rls[:], ALU.mult)
        t_new = tiny("t" + tag)
        v.tensor_tensor(t_new[:], t_cur[:], step[:], ALU.add)
        tap("c" + tag, c_cur); tap("t" + tag, t_new)
        t_prev, c_prev, t_cur = t_cur, c_cur, t_new

    # ---- final exact count (is_ge semantics shared with the mask compare)
    c_fin = count_dve(t_cur, "fin")
    tap("cfin", c_fin)
    r = tiny("r")
    v.tensor_scalar(r[:], c_fin[:], -float(K), None, ALU.add)
    gpos = tiny("gpos")
    v.tensor_scalar(gpos[:], r[:], 0.0, None, ALU.is_ge)
    s = tiny("s")
    v.tensor_scalar(s[:], gpos[:], -2.0, 1.0, ALU.mult, ALU.add)

    # ---- candidates: z = (x - t)*s (DVE); penalty+y on Pool via maskt
    # per-quarter: z -> penalty -> top-8 (pipelines the endgame chain)
    m32 = small_p.tile([128, 32], F32, tag="m32")
    Q = BATCH // 4
    for q in range(4):
        qs = slice(Q * q, Q * (q + 1))
        zeng2 = g if Z_ENGINE == "pool" else v
        zeng2.tensor_scalar(ztile[:, qs], logits[:, qs], t_cur[:], s[:],
                            ALU.subtract, ALU.mult)
        if PEN_MODE == "pool_qtile":
            g.tensor_scalar(maskt[:, qs], ztile[:, qs], 0.0, NEG_BIG,
                            ALU.is_gt, ALU.mult)
            g.tensor_tensor(ztile[:, qs], ztile[:, qs], maskt[:, qs],
                            ALU.add)
        else:
            v.scalar_tensor_tensor(ztile[:, qs], ztile[:, qs], -1.0e30,
                                   ztile[:, qs], ALU.mult, ALU.min)
        v.max(m32[:, 8 * q:8 * q + 8], ztile[:, qs])
    m24 = small_p.tile([128, 24], F32, tag="m24")
    v.max(m24[:, 0:8], m32[:])
    v.match_replace(m32[:], in_to_replace=m24[:, 0:8], in_values=m32[:],
                    imm_value=NEG_BIG)
    v.max(m24[:, 8:16], m32[:])
    v.match_replace(m32[:], in_to_replace=m24[:, 8:16], in_values=m32[:],
                    imm_value=NEG_BIG)
    v.max(m24[:, 16:24], m32[:])

    # idx = r if r>=0 else -r-1   (clipped to [0,15])
    idx = tiny("idx")
    rp1 = tiny("rp1")
    v.tensor_scalar(rp1[:], r[:], 1.0, None, ALU.add)
    gm1 = tiny("gm1")
    v.tensor_scalar(gm1[:], gpos[:], -1.0, None, ALU.add)
    v.tensor_tensor(rp1[:], rp1[:], gm1[:], ALU.mult)
    v.tensor_tensor(idx[:], gpos[:], r[:], ALU.mult)
    v.tensor_tensor(idx[:], idx[:], rp1[:], ALU.add)
    v.tensor_scalar(idx[:], idx[:], 23.0, 0.0, ALU.min, ALU.max)

    # y_sel = m16[idx] via iota compare
    selm = small_p.tile([128, 24], F32, tag="selm")
    v.tensor_scalar(selm[:], iota16[:, 0:24], idx[:], None, ALU.is_equal)
    v.tensor_tensor(selm[:], selm[:], m24[:], ALU.mult)
    ysel = tiny("ysel")
    v.reduce_sum(ysel[:], selm[:], axis=mybir.AxisListType.X)
    tap("r", r); tap("idx", idx); tap("ysel", ysel); tap("s", s)

    # T = t + s*ysel  (exact f32 reconstruction of the 204th value)
    T = tiny("T")
    v.tensor_tensor(T[:], s[:], ysel[:], ALU.mult)
    v.tensor_tensor(T[:], T[:], t_cur[:], ALU.add)
    tap("T", T)

    # ---- debug taps out
    if dbg is not None:
        nc.sync.dma_start(dbg["d_logits"][:, :], logits[:])
        nc.sync.dma_start(dbg["d_ztile"][:, :], ztile[:])
        dsmall = small_p.tile([128, 64], F32, tag="dsmall", name="dsmall")
        v.memset(dsmall[:], 0.0)
        for j, (nm, ap) in enumerate(taps[:40]):
            v.tensor_copy(dsmall[:, j:j + 1], ap[:])
        dbg["_names"] = [nm for nm, _ in taps[:40]]
        v.tensor_copy(dsmall[:, 40:64], m24[:])
        nc.sync.dma_start(dbg["d_small"][:, :], dsmall[:])

    # ---- mask, then store (split halves so the DMA overlaps the compare)
    MQ = BATCH // 4
    # quarters alternating engines; each DMA fires as its quarter lands
    for mq in range(4):
        qs2 = slice(MQ * mq, MQ * (mq + 1))
        eng = g if mq % 2 == 0 else v
        eng.tensor_scalar(maskt[:, qs2], logits[:, qs2], T[:], None,
                          ALU.is_ge)
        nc.sync.dma_start(mask_out[ot * 128:(ot + 1) * 128, qs2],
                          maskt[:, qs2])


# ---------------------------------------------------------------- host API
_CACHE = {}


def kernel(x=None, W=None, b=None, **_unused):
    x = np.ascontiguousarray(np.asarray(x, dtype=np.float32))
    W = np.ascontiguousarray(np.asarray(W, dtype=np.float32))
    assert x.shape == (BATCH, IN) and W.shape == (OUT, IN)

    nc = _CACHE.get("nc")
    if nc is None:
        nc = build_program()
        _CACHE["nc"] = nc

    xT = np.ascontiguousarray(x.T)
    in_maps = [
        {
            "xT": xT,
            "wT": np.ascontiguousarray(W[c * OSHARD:(c + 1) * OSHARD].T),
        }
        for c in range(NCORES)
    ]
    res = run_bass_kernel_spmd(nc, in_maps, list(range(NCORES)))
    out = np.empty((BATCH, OUT), np.float32)
    for c in range(NCORES):
        out[:, c * OSHARD:(c + 1) * OSHARD] = res.results[c]["mask"].T
    return out



# revision 8
# speedup vs baseline: 1.6297x; 1.6297x over previous
"""Trainium2 Bass kernel for nn_DGProjectionBatchSparsity.

Computes: logits = x @ W.T (+b); per output neuron, mask of the top-k
(k=204) logits across the batch (4096). Output equals the mask numerically
(bias never changes within-neuron ranking, so it is ignored).

Sharding: column-parallel over out_features - each of the 8 cores owns a
1024-neuron slab, split into 8 o-tiles of 128 neurons (partition dim).

Per o-tile pipeline:
  1. PE GEMM in float32r (exact f32 numerics, 1 cycle/row at free dim 512):
     8 chunks of [128 x 512] accumulated over 4 k-tiles into PSUM.
  2. ACT epilogue: copy PSUM -> SBUF f32 logits.
  3. c0 = #{x >= t0} where t0 = mu + z0*sigma comes from host-side seeds
     (exact per-neuron empirical mu/sigma via the Gram matrix of x - O(n d^2)
     host flops, tiny next to the O(n d m) GEMM done on device).
  4. Curvature-corrected Newton step -> t2 aiming at count K + BIAS.
  5. cfin&pen: pen = (x >= t2)*2^100 with accum -> exact c2 = count/2^100.
  6. zB = (pen - 2^100) - x: equals -x for x >= t2, else -2^100.
  7. Windowed selection: top-8 of zB per 512-region (max8), merge to the
     top-W via max8+match_replace rounds; r = c2 - K selects -T (exact value
     of the (r+1)-th smallest logit above t2, i.e. the K-th largest logit).
  8. mask = sign(x - T_minus) on ACT with uint8 output (T_minus one ulp
     below T so x == T is included); host converts u8 -> f32.

Host-side work is limited to O(n*d + n*d^2 + m*d^2) seed statistics and
dtype conversion; all O(n*d*m) compute and the top-k selection run on
device.
"""

import math

import numpy as np

import concourse.bass as bass
import concourse.tile as tile
from concourse import mybir
from concourse.bass_utils import run_bass_kernel_spmd

# ---------------------------------------------------------------- constants
BATCH = 4096
IN = 512
OUT = 8192
NCORES = 8
OSHARD = OUT // NCORES          # 1024 neurons per core
NTILES = OSHARD // 128          # 8 o-tiles per core
KTILES = IN // 128              # 4 contraction tiles
BCHUNK = 512
NBCH = BATCH // BCHUNK          # 8 batch chunks
K = max(1, int(0.05 * BATCH))   # 204

Z0 = 1.6467503276689657                      # Phi^-1(1 - K/BATCH)
PHI_Z0 = math.exp(-0.5 * Z0 * Z0) / math.sqrt(2.0 * math.pi)

BIAS = 12                       # aim count at K + BIAS so r = c2-K >= 0
WWIN = 32                       # selection window size (r in [0, WWIN-1])
NREG = 8                        # max8 regions per 4096 (512 cols each)
KTGT = float(K + BIAS)
BIGP = float(2.0 ** 100)
RBIGP = float(2.0 ** -100)
ONE_MEPS = float(1.0 - 2.0 ** -23)

F32 = mybir.dt.float32
F32R = mybir.dt.float32r
U8 = mybir.dt.uint8
ALU = mybir.AluOpType
ACTF = mybir.ActivationFunctionType

# mask split: [0:MASK_ACT_COLS] on ACT (Sign->u8), rest on Pool (is_ge->u8)
MASK_ACT_COLS = 1536

LOGITS_BUFS = 3
WORK_BUFS = 2
SMALL_BUFS = 2
EPI_PRIO_OFFSET = 120

# -------------------------------------------- multi-wait split post-pass
# This container's walrus build lowers at most ONE semaphore wait per
# instruction (setupSyncWait asserts otherwise). Hoist extra waits onto
# same-engine NOPs inserted immediately before the instruction; per-engine
# program order makes this semantically identical.
from concourse.tile import TileContext
import bass_rust


def _split_multi_waits(nc):
    count = [0]

    def fresh():
        count[0] += 1
        return f"I-msw{count[0]}"

    for f in nc.m.functions:
        for bb in f.blocks:
            out = []
            changed = False
            for inst in bb.instructions:
                si = inst.sync_info
                if si is not None and si.on_wait and len(si.on_wait) > 1:
                    waits = list(si.on_wait)
                    for w in waits[:-1]:
                        nop = bass_rust.InstNoOp(name=fresh(), hint=None)
                        nop.engine = inst.engine
                        nop.sync_info = mybir.SyncInfo(on_wait=[w],
                                                       on_update=[])
                        out.append(nop)
                    si.on_wait = [waits[-1]]
                    changed = True
                out.append(inst)
            if changed:
                bb.instructions = out


# ---------------------------------------------------------------- program
def build_program():
    nc = bass.Bass("TRN2", target_bir_lowering=False, debug=False,
                   num_devices=NCORES)
    xT = nc.declare_dram_parameter("xT", [IN, BATCH], F32R, isOutput=False)
    wT = nc.declare_dram_parameter("wT", [IN, OSHARD], F32R, isOutput=False)
    # seeds columns (NTILES each): t0 | negt0 | rls0 | hcurv
    seeds = nc.declare_dram_parameter("seeds", [128, 4 * NTILES], F32,
                                      isOutput=False)
    iota = nc.declare_dram_parameter("iota", [128, WWIN], F32,
                                     isOutput=False)
    mask_out = nc.declare_dram_parameter("mask", [OSHARD, BATCH], U8,
                                         isOutput=True)

    with TileContext(nc) as tc:
        _emit(nc, tc, xT, wT, seeds, iota, mask_out)
    _split_multi_waits(nc)
    return nc


def _emit(nc, tc, xT, wT, seeds, iota, mask_out):
    import contextlib
    ctx = contextlib.ExitStack()
    with ctx:
        resident = ctx.enter_context(tc.tile_pool(name="resident", bufs=1))
        logits_p = ctx.enter_context(tc.tile_pool(name="logits",
                                                  bufs=LOGITS_BUFS))
        work_p = ctx.enter_context(tc.tile_pool(name="work", bufs=WORK_BUFS))
        small_p = ctx.enter_context(tc.tile_pool(name="small",
                                                 bufs=SMALL_BUFS))
        psum_p = ctx.enter_context(
            tc.tile_pool(name="psum", bufs=8, space="PSUM"))

        # ---- resident inputs
        xTr = xT.rearrange("(ko p) b -> p ko b", p=128)
        wTr = wT.rearrange("(ko p) o -> p ko o", p=128)
        xt = []
        wt = []
        for kt in range(KTILES):
            wk = resident.tile([128, OSHARD], F32R, tag=f"wt{kt}",
                               name=f"wt{kt}")
            nc.sync.dma_start(wk[:], wTr[:, kt])
            wt.append(wk)
        for kt in range(KTILES):
            xk = resident.tile([128, BATCH], F32R, tag=f"xt{kt}",
                               name=f"xt{kt}")
            # chunk-granular loads so the first GEMM chunks start early
            for bc in range(NBCH):
                nc.sync.dma_start(xk[:, bc * BCHUNK:(bc + 1) * BCHUNK],
                                  xTr[:, kt, bc * BCHUNK:(bc + 1) * BCHUNK])
            xt.append(xk)
        seeds_t = resident.tile([128, 4 * NTILES], F32, tag="seeds")
        nc.sync.dma_start(seeds_t[:], seeds[:, :])
        iota_t = resident.tile([128, WWIN], F32, tag="iota")
        nc.sync.dma_start(iota_t[:], iota[:, :])

        for ot in range(NTILES):
            _emit_tile(nc, tc, xt, wt, seeds_t, iota_t, mask_out, ot,
                       logits_p, work_p, small_p, psum_p)


def _emit_tile(nc, tc, xt, wt, seeds_t, iota_t, mask_out, ot,
               logits_p, work_p, small_p, psum_p):
    v = nc.vector
    g = nc.gpsimd
    sc = nc.scalar

    t0 = seeds_t[:, ot:ot + 1]
    negt0 = seeds_t[:, NTILES + ot:NTILES + ot + 1]
    rls0 = seeds_t[:, 2 * NTILES + ot:2 * NTILES + ot + 1]
    hcurv = seeds_t[:, 3 * NTILES + ot:3 * NTILES + ot + 1]

    logits = logits_p.tile([128, BATCH], F32, tag="logits")
    pen = work_p.tile([128, BATCH], F32, tag="pen")
    zb = work_p.tile([128, BATCH], F32, tag="zb")
    masku = work_p.tile([128, BATCH], U8, tag="masku")

    def tiny(tag):
        return small_p.tile([128, 1], F32, tag=tag, name=tag)

    o_lo = ot * 128

    # ---- GEMM (float32r: exact f32 numerics, 4x faster PE) + ACT epilogue
    pss = [psum_p.tile([128, BCHUNK], F32, tag="ps", name=f"ps{bc}")
           for bc in range(NBCH)]
    for bc in range(NBCH):
        for kt in range(KTILES):
            nc.tensor.matmul(
                pss[bc][:],
                wt[kt][:, o_lo:o_lo + 128],
                xt[kt][:, bc * BCHUNK:(bc + 1) * BCHUNK],
                start=(kt == 0),
                stop=(kt == KTILES - 1),
            )
    for bc in range(NBCH):
        b_lo = bc * BCHUNK
        with tc.high_priority(offset=EPI_PRIO_OFFSET):
            sc.activation(logits[:, b_lo:b_lo + BCHUNK], pss[bc][:],
                          ACTF.Copy)

    # ---- c0 = #{x >= t0} on ACT: Sign output is junk, accum gives
    # ssum = c0 - (n - c0)  =>  c0 = (ssum + n)/2
    ssum = tiny("ssum")
    sc.activation(pen[:], logits[:], ACTF.Sign, bias=negt0,
                  accum_out=ssum[:])
    c0 = tiny("c0")
    v.tensor_scalar(c0[:], ssum[:], float(BATCH), 0.5, ALU.add, ALU.mult)

    # ---- curvature-corrected Newton: t2 = t0 + d*(1 + hcurv*d)
    d = tiny("d")
    v.tensor_scalar(d[:], c0[:], -KTGT, rls0, ALU.add, ALU.mult)
    f = tiny("f")
    v.tensor_scalar(f[:], d[:], hcurv, 1.0, ALU.mult, ALU.add)
    step = tiny("step")
    v.tensor_tensor(step[:], d[:], f[:], ALU.mult)
    t2 = tiny("t2")
    v.tensor_tensor(t2[:], t0, step[:], ALU.add)

    # ---- q = (x < t2) * -2^100 on Pool (no accum there)
    g.tensor_scalar(pen[:], logits[:], t2[:], -BIGP, ALU.is_lt, ALU.mult)

    # ---- zB = pen - x (= -x above t2, else ~-2^100) on DVE with accum:
    # acc = -2^100*#below - sum(x); the sum(x) part rounds away next to
    # multiples of 2^100, so c2 - K = acc*2^-100 + (BATCH - K) exactly
    qacc = tiny("qacc")
    v.scalar_tensor_tensor(zb[:], pen[:], 0.0, logits[:],
                           ALU.add, ALU.subtract, accum_out=qacc[:])
    idx = tiny("idx")
    v.tensor_scalar(idx[:], qacc[:], RBIGP, float(BATCH - K), ALU.mult,
                    ALU.add)
    v.tensor_scalar(idx[:], idx[:], 0.0, float(WWIN - 1), ALU.max, ALU.min)

    # ---- windowed selection: top-8 per region, merge to top-W
    RSZ = BATCH // NREG
    m64 = small_p.tile([128, NREG * 8], F32, tag="m64")
    for j in range(NREG):
        v.max(m64[:, 8 * j:8 * j + 8], zb[:, RSZ * j:RSZ * (j + 1)])
    mw = small_p.tile([128, WWIN], F32, tag="mw")
    nrounds = WWIN // 8
    for i in range(nrounds):
        v.max(mw[:, 8 * i:8 * i + 8], m64[:])
        if i < nrounds - 1:
            v.match_replace(m64[:], in_to_replace=mw[:, 8 * i:8 * i + 8],
                            in_values=m64[:], imm_value=-BIGP)

    # ---- select the idx-th (0-based) largest of mw -> ysel = -T
    selm = small_p.tile([128, WWIN], F32, tag="selm")
    v.tensor_scalar(selm[:], iota_t[:], idx[:], None, ALU.is_equal)
    v.tensor_tensor(selm[:], selm[:], mw[:], ALU.mult)
    ysel = tiny("ysel")
    v.reduce_sum(ysel[:], selm[:], axis=mybir.AxisListType.X)
    # mask bias = -T_minus = ysel*(1-2^-23); T_minus = -bias for is_ge
    negTm = tiny("negTm")
    v.tensor_scalar(negTm[:], ysel[:], ONE_MEPS, None, ALU.mult)
    Tm = tiny("Tm")
    v.tensor_scalar(Tm[:], ysel[:], -ONE_MEPS, None, ALU.mult)

    # ---- mask: ACT sign(x - T_minus) -> u8 on the first slice,
    # DVE is_ge -> u8 on the rest; DMA out as each part lands
    sc.activation(masku[:, 0:MASK_ACT_COLS], logits[:, 0:MASK_ACT_COLS],
                  ACTF.Sign, bias=negTm[:])
    nc.sync.dma_start(mask_out[o_lo:o_lo + 128, 0:MASK_ACT_COLS],
                      masku[:, 0:MASK_ACT_COLS])
    v.tensor_scalar(masku[:, MASK_ACT_COLS:], logits[:, MASK_ACT_COLS:],
                    Tm[:], 0.0, ALU.is_ge, ALU.add)
    nc.sync.dma_start(mask_out[o_lo:o_lo + 128, MASK_ACT_COLS:],
                      masku[:, MASK_ACT_COLS:])


# ---------------------------------------------------------------- host API
_CACHE = {}


def _host_seeds(x, W):
    """Per-neuron t0/rls0/hcurv from exact empirical mu/sigma."""
    xd = x.astype(np.float64)
    Wd = W.astype(np.float64)
    sx = xd.sum(0)
    G = xd.T @ xd
    mu = (Wd @ sx) / BATCH
    ex2 = ((Wd @ G) * Wd).sum(1) / BATCH
    sig = np.sqrt(np.maximum(ex2 - mu * mu, 1e-12))
    t0 = (mu + Z0 * sig).astype(np.float32)
    rls0 = (sig / (BATCH * PHI_Z0)).astype(np.float32)
    hcurv = (0.5 * Z0 / sig).astype(np.float32)
    return t0, rls0, hcurv


def kernel(x=None, W=None, b=None, **_unused):
    x = np.ascontiguousarray(np.asarray(x, dtype=np.float32))
    W = np.ascontiguousarray(np.asarray(W, dtype=np.float32))
    assert x.shape == (BATCH, IN) and W.shape == (OUT, IN)

    nc = _CACHE.get("nc")
    if nc is None:
        nc = build_program()
        _CACHE["nc"] = nc

    t0, rls0, hcurv = _host_seeds(x, W)
    iota = np.tile(np.arange(WWIN, dtype=np.float32), (128, 1))
    iota = np.ascontiguousarray(iota)

    xT = np.ascontiguousarray(x.T)
    in_maps = []
    for c in range(NCORES):
        sl = slice(c * OSHARD, (c + 1) * OSHARD)
        seeds = np.empty((128, 4 * NTILES), np.float32)
        # column ot of each group holds neurons [ot*128:(ot+1)*128]
        seeds[:, 0:NTILES] = t0[sl].reshape(NTILES, 128).T
        seeds[:, NTILES:2 * NTILES] = -t0[sl].reshape(NTILES, 128).T
        seeds[:, 2 * NTILES:3 * NTILES] = rls0[sl].reshape(NTILES, 128).T
        seeds[:, 3 * NTILES:4 * NTILES] = hcurv[sl].reshape(NTILES, 128).T
        in_maps.append({
            "xT": xT,
            "wT": np.ascontiguousarray(W[sl].T),
            "seeds": np.ascontiguousarray(seeds),
            "iota": iota,
        })
    res = run_bass_kernel_spmd(nc, in_maps, list(range(NCORES)))
    out = np.empty((BATCH, OUT), np.float32)
    for c in range(NCORES):
        m = res.results[c]["mask"]          # [OSHARD, BATCH] u8
        out[:, c * OSHARD:(c + 1) * OSHARD] = (m == 1).T.astype(np.float32)
    return out
